# revision 6
# baseline (speedup 1.0000x reference)
"""Biaffine edge attention on 8 Trainium2 NeuronCores.

out[b,i,j] = head[b,i,:] @ edge_U @ dep[b,j,:] + head[b,i,:]@w1 + dep[b,j,:]@w2 + b0

Sharding: data-parallel over batch (B=8, one batch per core).

Everything runs in bf16 (host-converted; rel err ~4.5e-3 vs the 2e-2 gate),
so the PE does the two 1024^3 matmul chains (54.6 us floor) plus only the
64 H block-transposes:

  - P arrives pre-transposed through the DMA XBAR (dma_start transpose=True,
    bf16-only). The XBAR is slow (~90GB/s effective, serial), so mm2 is
    ordered jh-outer to push the second half's deadline to ~52us.
  - H is transposed on the PE from chunked natural loads (the XBAR can't
    deliver HT before mm1 wants to start).
  - s_head fold: host adds v = U^-1 w1 to dep before quantizing, so mm2's
    sum_k T1T[k,i]*v[k] = head_i @ (U v) = s_head[i] comes out for free.
  - s_dep fold: T1T'[k,i] = T1T[k,i] + w2[k] on the PSUM->SBUF copy makes
    mm2 emit sum_k w2[k]*PT[k,j] = s_dep[j].
  - cross term w2.v is constant, folded with b0 into the epilogue bias col.

Engines: PE = 256 matmuls + 64 transposes; DVE = transpose/T1T' copies +
epilogue bias-adds; ACT = two transpose-copies + the XBAR queue; SP = load
+ store queue; GpSimd = first U column only.
"""

import numpy as np
import ml_dtypes

import concourse.bass as bass
import concourse.mybir as mybir
import concourse.tile as tile
from concourse import bacc
from concourse.bass_utils import run_bass_kernel_spmd
from concourse.masks import make_identity

B, S, D = 8, 1024, 1024
P = 128
DO = 8       # 1024 / 128
NH = 512     # matmul free-dim tile (one fp32 PSUM bank)
F32 = mybir.dt.float32
BF16 = mybir.dt.bfloat16
ADD = mybir.AluOpType.add
BF = ml_dtypes.bfloat16

_CACHE = {}


def build_nc():
    nc = bacc.Bacc(None, target_bir_lowering=False)

    head = nc.dram_tensor("head", [S, D], BF16, kind="ExternalInput")
    depv = nc.dram_tensor("depv", [S, D], BF16, kind="ExternalInput")
    # u_prep[kt, dd, do, kk] = U[do*128+dd, kt*128+kk]
    edge_u = nc.dram_tensor("edge_u", [DO, P, DO, P], BF16, kind="ExternalInput")
    # cols 0..7 = w2 reshaped [kk, kt]; col 8 = b0 - w2.v bias column
    w2bc = nc.dram_tensor("w2bc", [P, DO + 1], F32, kind="ExternalInput")
    out = nc.dram_tensor("out", [S, S], F32, kind="ExternalOutput")

    with tile.TileContext(nc) as tc:
        with (
            tc.tile_pool(name="const", bufs=1) as const,
            tc.tile_pool(name="big", bufs=1) as big,
            tc.tile_pool(name="stage", bufs=8) as stage,
            tc.tile_pool(name="outp", bufs=4) as outp,
            tc.tile_pool(name="tp_ps", bufs=2, space="PSUM") as tp_ps,
            tc.tile_pool(name="mm_ps", bufs=4, space="PSUM") as mm_ps,
        ):
            ident_raw = const.tile([P, P], F32)
            make_identity(nc, ident_raw)
            ident = const.tile([P, P], BF16)
            nc.vector.tensor_copy(ident[:], ident_raw[:])
            wb = const.tile([P, DO + 1], F32)

            u_sb = big.tile([P, DO, DO, P], BF16, tag="u")    # [dd, kt, do, kk]
            ht_sb = big.tile([P, DO, S], BF16, tag="ht")      # [dd, do, i]
            pt_sb = big.tile([P, DO, S], BF16, tag="pt")      # [kk, kt, j] (+v)
            t1t_sb = big.tile([P, DO, S], BF16, tag="t1t")    # [kk, kt, i]

            # ---------- DMA dispatch (per-queue FIFO; order = priority) -----
            # SP queue: small consts, H chunks + U columns interleaved by
            # consumption deadline, then the output stores.
            h_stage = [None] * DO

            def load_h(io):
                t = stage.tile([P, D], BF16, tag="stage")
                nc.sync.dma_start(t[:], head[io * P:(io + 1) * P, :])
                h_stage[io] = t

            nc.sync.dma_start(wb[:], w2bc[:])
            nc.gpsimd.dma_start(u_sb[:, 0], edge_u[0])
            for io in range(4):
                load_h(io)
            for kt in range(1, 4):
                nc.sync.dma_start(u_sb[:, kt], edge_u[kt])
                load_h(kt + 3)
            load_h(7)
            for kt in range(4, DO):
                nc.sync.dma_start(u_sb[:, kt], edge_u[kt])

            # ---------- PE helpers ------------------------------------------
            def tpose(io):
                """Transpose stage io (8 [128,128] blocks) into one PSUM tile."""
                ps = tp_ps.tile([P, S], BF16, tag="tp")
                for do in range(DO):
                    nc.tensor.transpose(
                        ps[:, do * P:(do + 1) * P],
                        h_stage[io][:, do * P:(do + 1) * P],
                        ident[:],
                    )
                return ps

            def tp_copy(eng, io, ps):
                dst = ht_sb[:, :, io * P:(io + 1) * P]
                src = ps[:].rearrange("p (q c) -> p q c", q=DO)
                if eng == "act":
                    nc.scalar.copy(dst, src)
                else:
                    nc.vector.tensor_copy(dst, src)

            def mm1_group(kt, ih, quarter=None):
                if quarter is None:
                    cs = slice(ih * NH, (ih + 1) * NH)
                    n = NH
                else:
                    cs = slice(quarter * 256, (quarter + 1) * 256)
                    n = 256
                ps = mm_ps.tile([P, NH], F32, tag="mm")
                for do in range(DO):
                    nc.tensor.matmul(
                        ps[:, 0:n],
                        u_sb[:, kt, do, :],
                        ht_sb[:, do, cs],
                        start=(do == 0),
                        stop=(do == DO - 1),
                    )
                nc.vector.tensor_scalar(
                    t1t_sb[:, kt, cs], ps[:, 0:n], wb[:, kt:kt + 1], None, ADD,
                )

            # ---------- phase A: transpose H io 0..3 ------------------------
            # copies: io0,1 on DVE; io2,3 on ACT (ahead of the XBAR SEQ ops)
            for io in range(4):
                ps = tpose(io)
                tp_copy("dve" if io < 2 else "act", io, ps)

            # ACT queue now takes the two XBAR P transposes (dep arrives
            # pre-shifted by +v from the host, so pt_sb is PT' directly).
            for jh in range(2):
                nc.scalar.dma_start(
                    pt_sb[:, :, jh * NH:(jh + 1) * NH],
                    depv[jh * NH:(jh + 1) * NH, :],
                    transpose=True,
                )

            # ---------- phase B: mm1 ih0 (kt0 split for earlier start), -----
            # H transposes io4..7 interleaved (copies on DVE)
            mm1_group(0, 0, quarter=0)
            mm1_group(0, 0, quarter=1)
            for kt in range(1, DO):
                if kt <= 4:
                    ps = tpose(kt + 3)
                    tp_copy("dve", kt + 3, ps)
                mm1_group(kt, 0)

            # ---------- phase C: mm1 ih1 ------------------------------------
            for kt in range(DO):
                mm1_group(kt, 1)

            # ---------- phase D: mm2, jh-outer; epilogue on DVE -------------
            for jh in range(2):
                for it in range(DO):
                    ps = mm_ps.tile([P, NH], F32, tag="mm")
                    for kt in range(DO):
                        nc.tensor.matmul(
                            ps[:],
                            t1t_sb[:, kt, it * P:(it + 1) * P],
                            pt_sb[:, kt, jh * NH:(jh + 1) * NH],
                            start=(kt == 0),
                            stop=(kt == DO - 1),
                        )
                    ot = outp.tile([P, NH], F32, tag="out")
                    last = (it == DO - 1 and jh == 1)
                    split = 4 if last else 1
                    w = NH // split
                    for s in range(split):
                        sl = slice(s * w, (s + 1) * w)
                        nc.vector.tensor_scalar(
                            ot[:, sl], ps[:, sl], wb[:, DO:DO + 1], None, ADD,
                        )
                        eng = nc.scalar if (last and s % 2 == 1) else nc.sync
                        eng.dma_start(
                            out[it * P:(it + 1) * P,
                                jh * NH + s * w:jh * NH + (s + 1) * w],
                            ot[:, sl],
                        )

    nc.compile()
    return nc


def _get_nc():
    if "nc" not in _CACHE:
        _CACHE["nc"] = build_nc()
    return _CACHE["nc"]


def _in_maps(head, dep, edge_U, edge_W, edge_b):
    head = np.asarray(head, dtype=np.float32)
    dep = np.asarray(dep, dtype=np.float32)
    U = np.asarray(edge_U, dtype=np.float32)
    w = np.asarray(edge_W, dtype=np.float32).reshape(-1)
    w1, w2 = w[:D], w[D:]
    b0 = float(np.asarray(edge_b, dtype=np.float32).reshape(-1)[0])

    Ub = U.astype(BF)
    # v = U^-1 w1 against the bf16-rounded U the device actually uses, so
    # sum_k T1T[k,i] v[k] reproduces head_i @ w1 up to bf16 noise. The shift
    # is applied to dep on the host: PT'[k,j] = dep[j,k] + v[k].
    v = np.linalg.solve(Ub.astype(np.float64), w1.astype(np.float64))
    v32 = v.astype(np.float32)

    u_prep = np.ascontiguousarray(
        Ub.reshape(DO, P, DO, P).transpose(2, 1, 0, 3)
    )
    w2bc = np.empty((P, DO + 1), dtype=np.float32)
    w2bc[:, :DO] = w2.reshape(DO, P).T
    w2bc[:, DO] = b0 - float(w2.astype(np.float64) @ v)

    maps = []
    for b in range(B):
        maps.append({
            "head": np.ascontiguousarray(head[b]).astype(BF),
            "depv": (dep[b] + v32[None, :]).astype(BF),
            "edge_u": u_prep,
            "w2bc": w2bc,
        })
    return maps


def kernel(head, dep, edge_U, edge_W, edge_b, **run_kwargs):
    nc = _get_nc()
    maps = _in_maps(head, dep, edge_U, edge_W, edge_b)
    res = run_bass_kernel_spmd(nc, maps, core_ids=list(range(B)), **run_kwargs)
    out = np.stack([np.asarray(res.results[c]["out"]) for c in range(B)], axis=0)
    if run_kwargs:
        _CACHE["last_result"] = res
    return out


# revision 8
# speedup vs baseline: 1.2057x; 1.2057x over previous
"""Biaffine edge attention on 8 Trainium2 NeuronCores.

out[b,i,j] = head[b,i,:] @ edge_U @ dep[b,j,:] + head[b,i,:]@w1 + dep[b,j,:]@w2 + b0

Sharding: data-parallel over batch (B=8, one batch per core).

Everything runs in bf16 (host-converted; rel err ~4.5e-3 vs the 2e-2 gate),
so the PE does the two 1024^3 matmul chains (54.6 us floor) plus only the
64 H block-transposes:

  - P arrives pre-transposed through the DMA XBAR (dma_start transpose=True,
    bf16-only). The XBAR is slow (~90GB/s effective, serial), so mm2 is
    ordered jh-outer to push the second half's deadline to ~52us.
  - H is transposed on the PE from chunked natural loads (the XBAR can't
    deliver HT before mm1 wants to start).
  - s_head fold: host adds v = U^-1 w1 to dep before quantizing, so mm2's
    sum_k T1T[k,i]*v[k] = head_i @ (U v) = s_head[i] comes out for free.
  - s_dep fold: T1T'[k,i] = T1T[k,i] + w2[k] on the PSUM->SBUF copy makes
    mm2 emit sum_k w2[k]*PT[k,j] = s_dep[j].
  - cross term w2.v is constant, folded with b0 into the epilogue bias col.

Engines: PE = 256 matmuls + 64 transposes; DVE = transpose/T1T' copies +
epilogue bias-adds; ACT = two transpose-copies + the XBAR queue; SP = load
+ store queue; GpSimd = first U column only.
"""

import numpy as np
import ml_dtypes

import concourse.bass as bass
import concourse.mybir as mybir
import concourse.tile as tile
from concourse import bacc
from concourse.bass_utils import run_bass_kernel_spmd
from concourse.masks import make_identity

B, S, D = 8, 1024, 1024
P = 128
DO = 8       # 1024 / 128
NH = 512     # matmul free-dim tile (one fp32 PSUM bank)
F32 = mybir.dt.float32
BF16 = mybir.dt.bfloat16
ADD = mybir.AluOpType.add
BF = ml_dtypes.bfloat16

_CACHE = {}


def build_nc():
    nc = bacc.Bacc(None, target_bir_lowering=False)

    head = nc.dram_tensor("head", [S, D], BF16, kind="ExternalInput")
    depv = nc.dram_tensor("depv", [S, D], BF16, kind="ExternalInput")
    # u_prep[kt, dd, do, kk] = U[do*128+dd, kt*128+kk]
    edge_u = nc.dram_tensor("edge_u", [DO, P, DO, P], BF16, kind="ExternalInput")
    # cols 0..7 = w2 reshaped [kk, kt]; col 8 = b0 - w2.v bias column
    w2bc = nc.dram_tensor("w2bc", [P, DO + 1], F32, kind="ExternalInput")
    out = nc.dram_tensor("out", [S, S], F32, kind="ExternalOutput")

    with tile.TileContext(nc) as tc:
        with (
            tc.tile_pool(name="const", bufs=1) as const,
            tc.tile_pool(name="big", bufs=1) as big,
            tc.tile_pool(name="stage", bufs=8) as stage,
            tc.tile_pool(name="outp", bufs=4) as outp,
            tc.tile_pool(name="tp_ps", bufs=2, space="PSUM") as tp_ps,
            tc.tile_pool(name="mm_ps", bufs=4, space="PSUM") as mm_ps,
        ):
            ident_raw = const.tile([P, P], F32)
            make_identity(nc, ident_raw)
            ident = const.tile([P, P], BF16)
            nc.vector.tensor_copy(ident[:], ident_raw[:])
            wb = const.tile([P, DO + 1], F32)

            u_sb = big.tile([P, DO, DO, P], BF16, tag="u")    # [dd, kt, do, kk]
            ht_sb = big.tile([P, DO, S], BF16, tag="ht")      # [dd, do, i]
            pt_sb = big.tile([P, DO, S], BF16, tag="pt")      # [kk, kt, j] (+v)
            t1t_sb = big.tile([P, DO, S], BF16, tag="t1t")    # [kk, kt, i]

            # ---------- DMA dispatch (per-queue FIFO; order = priority) -----
            # SP queue: small consts, H chunks + U columns interleaved by
            # consumption deadline, then the output stores.
            h_stage = [None] * DO

            def load_h(io):
                t = stage.tile([P, D], BF16, tag="stage")
                nc.sync.dma_start(t[:], head[io * P:(io + 1) * P, :])
                h_stage[io] = t

            nc.sync.dma_start(wb[:], w2bc[:])
            nc.gpsimd.dma_start(u_sb[:, 0], edge_u[0])
            for io in range(4):
                load_h(io)
            for kt in range(1, 4):
                nc.sync.dma_start(u_sb[:, kt], edge_u[kt])
                load_h(kt + 3)
            load_h(7)
            for kt in range(4, DO):
                nc.sync.dma_start(u_sb[:, kt], edge_u[kt])

            # ---------- PE helpers ------------------------------------------
            def tpose(io):
                """Transpose stage io (8 [128,128] blocks) into one PSUM tile."""
                ps = tp_ps.tile([P, S], BF16, tag="tp")
                for do in range(DO):
                    nc.tensor.transpose(
                        ps[:, do * P:(do + 1) * P],
                        h_stage[io][:, do * P:(do + 1) * P],
                        ident[:],
                    )
                return ps

            def tp_copy(eng, io, ps):
                dst = ht_sb[:, :, io * P:(io + 1) * P]
                src = ps[:].rearrange("p (q c) -> p q c", q=DO)
                if eng == "act":
                    nc.scalar.copy(dst, src)
                else:
                    nc.vector.tensor_copy(dst, src)

            def mm1_group(kt, ih, quarter=None):
                if quarter is None:
                    cs = slice(ih * NH, (ih + 1) * NH)
                    n = NH
                else:
                    cs = slice(quarter * 256, (quarter + 1) * 256)
                    n = 256
                ps = mm_ps.tile([P, NH], F32, tag="mm")
                for do in range(DO):
                    nc.tensor.matmul(
                        ps[:, 0:n],
                        u_sb[:, kt, do, :],
                        ht_sb[:, do, cs],
                        start=(do == 0),
                        stop=(do == DO - 1),
                    )
                nc.vector.tensor_scalar(
                    t1t_sb[:, kt, cs], ps[:, 0:n], wb[:, kt:kt + 1], None, ADD,
                )

            # ---------- phase A: transpose H io 0..3 ------------------------
            for io in range(4):
                ps = tpose(io)
                tp_copy("dve", io, ps)

            # The XBAR P transposes monopolize the shared HWDGE generator for
            # their whole transfer, starving every queued load. Gate them on
            # the last input load (u7) with a tiny WAW-dependency write into
            # pt_sb so the scheduler cannot hoist them earlier. (dep arrives
            # pre-shifted by +v from the host, so pt_sb is PT' directly.)
            for jh in range(2):
                nc.vector.tensor_copy(
                    pt_sb[:, 0, jh * NH:jh * NH + 1],
                    u_sb[:, DO - 1, 0, 0:1],
                )
                nc.scalar.dma_start(
                    pt_sb[:, :, jh * NH:(jh + 1) * NH],
                    depv[jh * NH:(jh + 1) * NH, :],
                    transpose=True,
                )

            # ---------- phase B: mm1 ih0 (kt0 split for earlier start), -----
            # H transposes io4..7 interleaved (copies on DVE)
            mm1_group(0, 0, quarter=0)
            mm1_group(0, 0, quarter=1)
            for kt in range(1, DO):
                if kt <= 4:
                    ps = tpose(kt + 3)
                    tp_copy("dve", kt + 3, ps)
                mm1_group(kt, 0)

            # ---------- phase C: mm1 ih1 ------------------------------------
            for kt in range(DO):
                mm1_group(kt, 1)

            # ---------- phase D: mm2, jh-outer; epilogue on DVE -------------
            for jh in range(2):
                for it in range(DO):
                    ps = mm_ps.tile([P, NH], F32, tag="mm")
                    for kt in range(DO):
                        nc.tensor.matmul(
                            ps[:],
                            t1t_sb[:, kt, it * P:(it + 1) * P],
                            pt_sb[:, kt, jh * NH:(jh + 1) * NH],
                            start=(kt == 0),
                            stop=(kt == DO - 1),
                        )
                    ot = outp.tile([P, NH], F32, tag="out")
                    last = (it == DO - 1 and jh == 1)
                    split = 4 if last else 1
                    w = NH // split
                    for s in range(split):
                        sl = slice(s * w, (s + 1) * w)
                        nc.vector.tensor_scalar(
                            ot[:, sl], ps[:, sl], wb[:, DO:DO + 1], None, ADD,
                        )
                        eng = nc.scalar if (last and s % 2 == 1) else nc.sync
                        eng.dma_start(
                            out[it * P:(it + 1) * P,
                                jh * NH + s * w:jh * NH + (s + 1) * w],
                            ot[:, sl],
                        )

    nc.compile()
    return nc


def _get_nc():
    if "nc" not in _CACHE:
        _CACHE["nc"] = build_nc()
    return _CACHE["nc"]


def _in_maps(head, dep, edge_U, edge_W, edge_b):
    head = np.asarray(head, dtype=np.float32)
    dep = np.asarray(dep, dtype=np.float32)
    U = np.asarray(edge_U, dtype=np.float32)
    w = np.asarray(edge_W, dtype=np.float32).reshape(-1)
    w1, w2 = w[:D], w[D:]
    b0 = float(np.asarray(edge_b, dtype=np.float32).reshape(-1)[0])

    Ub = U.astype(BF)
    # v = U^-1 w1 against the bf16-rounded U the device actually uses, so
    # sum_k T1T[k,i] v[k] reproduces head_i @ w1 up to bf16 noise. The shift
    # is applied to dep on the host: PT'[k,j] = dep[j,k] + v[k].
    v = np.linalg.solve(Ub.astype(np.float64), w1.astype(np.float64))
    v32 = v.astype(np.float32)

    u_prep = np.ascontiguousarray(
        Ub.reshape(DO, P, DO, P).transpose(2, 1, 0, 3)
    )
    w2bc = np.empty((P, DO + 1), dtype=np.float32)
    w2bc[:, :DO] = w2.reshape(DO, P).T
    w2bc[:, DO] = b0 - float(w2.astype(np.float64) @ v)

    maps = []
    for b in range(B):
        maps.append({
            "head": np.ascontiguousarray(head[b]).astype(BF),
            "depv": (dep[b] + v32[None, :]).astype(BF),
            "edge_u": u_prep,
            "w2bc": w2bc,
        })
    return maps


def kernel(head, dep, edge_U, edge_W, edge_b, **run_kwargs):
    nc = _get_nc()
    maps = _in_maps(head, dep, edge_U, edge_W, edge_b)
    res = run_bass_kernel_spmd(nc, maps, core_ids=list(range(B)), **run_kwargs)
    out = np.stack([np.asarray(res.results[c]["out"]) for c in range(B)], axis=0)
    if run_kwargs:
        _CACHE["last_result"] = res
    return out


# revision 9
# speedup vs baseline: 1.2111x; 1.0044x over previous
"""Biaffine edge attention on 8 Trainium2 NeuronCores.

out[b,i,j] = head[b,i,:] @ edge_U @ dep[b,j,:] + head[b,i,:]@w1 + dep[b,j,:]@w2 + b0

Sharding: data-parallel over batch (B=8, one batch per core).

Everything runs in bf16 (host-converted; rel err ~4.5e-3 vs the 2e-2 gate),
so the PE does the two 1024^3 matmul chains (54.6 us floor) plus only the
64 H block-transposes:

  - P arrives pre-transposed through the DMA XBAR (dma_start transpose=True,
    bf16-only). The XBAR monopolizes the shared HWDGE generator for its
    whole transfer (~10us/MB), so both XBAR ops are gated on the last input
    load (u7) via tiny WAW-dependency writes into pt_sb.
  - H is transposed on the PE from chunked natural loads (the XBAR cannot
    deliver HT before mm1 wants to start).
  - s_head fold: host adds v = U^-1 w1 to dep before quantizing, so mm2's
    sum_k T1T[k,i]*v[k] = head_i @ (U v) = s_head[i] comes out for free.
  - s_dep fold: T1T'[k,i] = T1T[k,i] + w2[k] on the PSUM->SBUF copy makes
    mm2 emit sum_k w2[k]*PT[k,j] = s_dep[j].
  - cross term w2.v is constant, folded with b0 into the epilogue bias col.

PSUM tiles are [128,1024] (2 banks) so only 8 mm1 + 8 mm2 + 8 tp pool
allocations exist — the end-of-kernel semaphore-drain cascade is
proportional to allocation count. mm2 runs it-outer producing full
[128,1024] output rows (8 stores).
"""

import numpy as np
import ml_dtypes

import concourse.bass as bass
import concourse.mybir as mybir
import concourse.tile as tile
from concourse import bacc
from concourse.bass_utils import run_bass_kernel_spmd
from concourse.masks import make_identity

B, S, D = 8, 1024, 1024
P = 128
DO = 8       # 1024 / 128
NH = 512     # one fp32 PSUM bank / half-chain width
F32 = mybir.dt.float32
BF16 = mybir.dt.bfloat16
ADD = mybir.AluOpType.add
BF = ml_dtypes.bfloat16

_CACHE = {}


def build_nc():
    nc = bacc.Bacc(None, target_bir_lowering=False)

    head = nc.dram_tensor("head", [S, D], BF16, kind="ExternalInput")
    depv = nc.dram_tensor("depv", [S, D], BF16, kind="ExternalInput")
    # u_prep[kt, dd, do, kk] = U[do*128+dd, kt*128+kk]
    edge_u = nc.dram_tensor("edge_u", [DO, P, DO, P], BF16, kind="ExternalInput")
    # cols 0..7 = w2 reshaped [kk, kt]; col 8 = b0 - w2.v bias column
    w2bc = nc.dram_tensor("w2bc", [P, DO + 1], F32, kind="ExternalInput")
    out = nc.dram_tensor("out", [S, S], F32, kind="ExternalOutput")

    with tile.TileContext(nc) as tc:
        with (
            tc.tile_pool(name="const", bufs=1) as const,
            tc.tile_pool(name="big", bufs=1) as big,
            tc.tile_pool(name="stage", bufs=8) as stage,
            tc.tile_pool(name="outp", bufs=2) as outp,
            tc.tile_pool(name="tp_ps", bufs=2, space="PSUM") as tp_ps,
            tc.tile_pool(name="mm_ps", bufs=3, space="PSUM") as mm_ps,
        ):
            ident_raw = const.tile([P, P], F32)
            make_identity(nc, ident_raw)
            ident = const.tile([P, P], BF16)
            nc.vector.tensor_copy(ident[:], ident_raw[:])
            wb = const.tile([P, DO + 1], F32)

            u_sb = big.tile([P, DO, DO, P], BF16, tag="u")    # [dd, kt, do, kk]
            ht_sb = big.tile([P, DO, S], BF16, tag="ht")      # [dd, do, i]
            pt_sb = big.tile([P, DO, S], BF16, tag="pt")      # [kk, kt, j] (+v)
            t1t_sb = big.tile([P, DO, S], BF16, tag="t1t")    # [kk, kt, i]

            # ---------- DMA dispatch (per-queue FIFO; order = priority) -----
            h_stage = [None] * DO

            def load_h(io):
                t = stage.tile([P, D], BF16, tag="stage")
                nc.sync.dma_start(t[:], head[io * P:(io + 1) * P, :])
                h_stage[io] = t

            nc.gpsimd.dma_start(u_sb[:, 0], edge_u[0])
            for io in range(4):
                load_h(io)
            nc.sync.dma_start(wb[:], w2bc[:])
            for kt in range(1, 4):
                nc.sync.dma_start(u_sb[:, kt], edge_u[kt])
                load_h(kt + 3)
            load_h(7)
            for kt in range(4, DO):
                nc.sync.dma_start(u_sb[:, kt], edge_u[kt])

            # ---------- phase A: transpose all of H on the PE ---------------
            def tpose(io):
                ps = tp_ps.tile([P, S], BF16, tag="tp")
                for do in range(DO):
                    nc.tensor.transpose(
                        ps[:, do * P:(do + 1) * P],
                        h_stage[io][:, do * P:(do + 1) * P],
                        ident[:],
                    )
                dst = ht_sb[:, :, io * P:(io + 1) * P]
                src = ps[:].rearrange("p (q c) -> p q c", q=DO)
                if io % 2 == 0:
                    nc.vector.tensor_copy(dst, src)
                else:
                    nc.scalar.copy(dst, src)

            for io in range(DO):
                tpose(io)

            # XBAR P transposes, gated on the last input load (u7) with WAW
            # dummy writes so the scheduler can't start them while input
            # loads still need the HWDGE. (dep is pre-shifted by +v on the
            # host, so pt_sb receives PT' directly.)
            for jh in range(2):
                nc.vector.tensor_copy(
                    pt_sb[:, 0, jh * NH:jh * NH + 1],
                    u_sb[:, DO - 1, 0, 0:1],
                )
                nc.scalar.dma_start(
                    pt_sb[:, :, jh * NH:(jh + 1) * NH],
                    depv[jh * NH:(jh + 1) * NH, :],
                    transpose=True,
                )

            # ---------- phase B: mm1, one [128,1024] PSUM tile per kt -------
            # T1T[k,i] = sum_d U[d,k] HT[d,i]; +w2[k] fold on the copies
            for kt in range(DO):
                ps = mm_ps.tile([P, S], F32, tag="mm")
                for ih in range(2):
                    cs = slice(ih * NH, (ih + 1) * NH)
                    for do in range(DO):
                        nc.tensor.matmul(
                            ps[:, cs],
                            u_sb[:, kt, do, :],
                            ht_sb[:, do, cs],
                            start=(do == 0),
                            stop=(do == DO - 1),
                        )
                    nc.vector.tensor_scalar(
                        t1t_sb[:, kt, cs], ps[:, cs], wb[:, kt:kt + 1],
                        None, ADD,
                    )

            # ---------- phase C: mm2 it-outer, full-row stores --------------
            for it in range(DO):
                ps = mm_ps.tile([P, S], F32, tag="mm")
                for jh in range(2):
                    cs = slice(jh * NH, (jh + 1) * NH)
                    for kt in range(DO):
                        nc.tensor.matmul(
                            ps[:, cs],
                            t1t_sb[:, kt, it * P:(it + 1) * P],
                            pt_sb[:, kt, cs],
                            start=(kt == 0),
                            stop=(kt == DO - 1),
                        )
                ot = outp.tile([P, S], F32, tag="out")
                last = (it == DO - 1)
                split = 4 if last else 1
                w = S // split
                for s in range(split):
                    sl = slice(s * w, (s + 1) * w)
                    nc.vector.tensor_scalar(
                        ot[:, sl], ps[:, sl], wb[:, DO:DO + 1], None, ADD,
                    )
                    eng = nc.scalar if (last and s % 2 == 1) else nc.sync
                    eng.dma_start(
                        out[it * P:(it + 1) * P, sl], ot[:, sl],
                    )

    nc.compile()
    return nc


def _get_nc():
    if "nc" not in _CACHE:
        _CACHE["nc"] = build_nc()
    return _CACHE["nc"]


def _in_maps(head, dep, edge_U, edge_W, edge_b):
    head = np.asarray(head, dtype=np.float32)
    dep = np.asarray(dep, dtype=np.float32)
    U = np.asarray(edge_U, dtype=np.float32)
    w = np.asarray(edge_W, dtype=np.float32).reshape(-1)
    w1, w2 = w[:D], w[D:]
    b0 = float(np.asarray(edge_b, dtype=np.float32).reshape(-1)[0])

    Ub = U.astype(BF)
    # v = U^-1 w1 against the bf16-rounded U the device actually uses, so
    # sum_k T1T[k,i] v[k] reproduces head_i @ w1 up to bf16 noise. The shift
    # is applied to dep on the host: PT'[k,j] = dep[j,k] + v[k].
    v = np.linalg.solve(Ub.astype(np.float64), w1.astype(np.float64))
    v32 = v.astype(np.float32)

    u_prep = np.ascontiguousarray(
        Ub.reshape(DO, P, DO, P).transpose(2, 1, 0, 3)
    )
    w2bc = np.empty((P, DO + 1), dtype=np.float32)
    w2bc[:, :DO] = w2.reshape(DO, P).T
    w2bc[:, DO] = b0 - float(w2.astype(np.float64) @ v)

    maps = []
    for b in range(B):
        maps.append({
            "head": np.ascontiguousarray(head[b]).astype(BF),
            "depv": (dep[b] + v32[None, :]).astype(BF),
            "edge_u": u_prep,
            "w2bc": w2bc,
        })
    return maps


def kernel(head, dep, edge_U, edge_W, edge_b, **run_kwargs):
    nc = _get_nc()
    maps = _in_maps(head, dep, edge_U, edge_W, edge_b)
    res = run_bass_kernel_spmd(nc, maps, core_ids=list(range(B)), **run_kwargs)
    out = np.stack([np.asarray(res.results[c]["out"]) for c in range(B)], axis=0)
    if run_kwargs:
        _CACHE["last_result"] = res
    return out
